# revision 23
# baseline (speedup 1.0000x reference)
"""Trainium2 Bass kernel for AimNet2Core message-passing block.

Strategy:
  - a_j = E[idx_j] depends only on the destination atom, so the radial branch
    collapses to  radial_emb[n] = E[n] * deg[n],  radial_q[n] = q[n] * deg[n]
    with deg[n] = segment_sum(gs.sum(-1)).
  - The vector branch uses the Gram identity
        vec_emb[n,h] = sum_{g,g'} S[n,g,g'] T[n,g,h] T[n,g',h]
    with S[n] = segment_sum_p(gv[p]^T gv[p])  (16x16 per pair, summed over d=3)
    and T = E @ agh  computed per atom (not per pair).
  - Host shards pairs by owner atom (sort by idx_j): core c owns atoms
    [6250c, 6250(c+1)).  Within a core, atoms are tiled into 49 windows of
    128; each window's pairs are processed in chunks of 128 and segment-summed
    via a one-hot matmul accumulated in PSUM.  No collectives needed.
  - MLP is computed per window entirely on-chip; output is [66, atoms].
"""
import math
import numpy as np
import ml_dtypes

import concourse.bass as bass
import concourse.bacc as bacc
import concourse.mybir as mybir
import concourse.tile as tile
from concourse.bass_utils import run_bass_kernel_spmd

N_ATOMS = 50000
N_CORES = 8
NA = N_ATOMS // N_CORES      # 6250
NW = 49                      # atom windows per core
NAP = NW * 128               # 6272 padded atoms per core
G = 16
V = 32
F = 64
H1 = 128
OUT_F = 66
IN_EFF = 97                  # 64 + 32 + 1 effective msg features (vec_q==0)

FP32 = mybir.dt.float32
BF16 = mybir.dt.bfloat16


def _host_prep(inputs):
    idx = np.asarray(inputs["pair_indices"][1], dtype=np.int64)
    P = idx.shape[0]
    order = np.argsort(idx, kind="stable")
    sidx = idx[order]
    gs_s = np.ascontiguousarray(np.asarray(inputs["gs"], np.float32)[order])
    gv_s = np.ascontiguousarray(
        np.asarray(inputs["gv"], np.float32)[order].reshape(P, 48))
    bounds = np.searchsorted(sidx, [NA * c for c in range(N_CORES + 1)])

    percore = []
    cw = np.ones(NW, np.int64)
    for c in range(N_CORES):
        lo, hi = bounds[c], bounds[c + 1]
        lidx = (sidx[lo:hi] - NA * c).astype(np.int64)
        w = lidx >> 7
        cnt = np.bincount(w, minlength=NW)
        cw = np.maximum(cw, (cnt + 127) // 128)
        percore.append((lo, hi, lidx, w, cnt))

    woff2 = np.concatenate([[0], np.cumsum(cw)])  # chunk offsets per window
    nslot = int(woff2[-1]) * 128
    cores = []
    for c in range(N_CORES):
        lo, hi, lidx, w, cnt = percore[c]
        gs_p = np.zeros((nslot, 16), ml_dtypes.bfloat16)
        gv_p = np.zeros((nslot, 48), ml_dtypes.bfloat16)
        ix_p = np.zeros((nslot, 1), ml_dtypes.bfloat16)
        woff = np.concatenate([[0], np.cumsum(cnt)[:-1]])
        pos_in_w = np.arange(len(lidx)) - woff[w]
        slot = woff2[w] * 128 + pos_in_w
        gs_p[slot] = gs_s[lo:hi]
        gv_p[slot] = gv_s[lo:hi]
        ix_p[slot, 0] = (lidx & 127).astype(ml_dtypes.bfloat16)
        cores.append((gs_p, gv_p, ix_p))
    return cores, [int(x) for x in cw]


def _build_nc(cw, sim_safe=False):
    # sim_safe: CoreSim doesn't implement Gelu; Relu is timing-identical
    # for cost-model profiling runs.
    act_func = (mybir.ActivationFunctionType.Relu if sim_safe
                else mybir.ActivationFunctionType.Gelu)
    nc = bacc.Bacc()
    CM = max(cw)
    woff2 = [0]
    for x in cw:
        woff2.append(woff2[-1] + x)
    nslot = woff2[-1] * 128

    gs_d = nc.dram_tensor("gsp", [nslot, 16], BF16, kind="ExternalInput")
    gv_d = nc.dram_tensor("gvp", [nslot, 48], BF16, kind="ExternalInput")
    ix_d = nc.dram_tensor("ixp", [nslot, 1], BF16, kind="ExternalInput")
    elocr_d = nc.dram_tensor("elocr", [128, NW * 64], FP32, kind="ExternalInput")
    qr_d = nc.dram_tensor("qr", [128, NW], FP32, kind="ExternalInput")
    agh_d = nc.dram_tensor("aghr", [64, 512], FP32, kind="ExternalInput")
    w1_d = nc.dram_tensor("w1e", [IN_EFF, H1], FP32, kind="ExternalInput")
    w2_d = nc.dram_tensor("w2", [H1, H1], FP32, kind="ExternalInput")
    w3_d = nc.dram_tensor("w3", [H1, OUT_F], FP32, kind="ExternalInput")
    b1_d = nc.dram_tensor("b1c", [H1, 1], FP32, kind="ExternalInput")
    b2_d = nc.dram_tensor("b2c", [H1, 1], FP32, kind="ExternalInput")
    b3_d = nc.dram_tensor("b3c", [OUT_F, 1], FP32, kind="ExternalInput")
    iota_d = nc.dram_tensor("iota", [128, 128, CM], BF16, kind="ExternalInput")
    ident_d = nc.dram_tensor("ident", [128, 128], FP32, kind="ExternalInput")
    out_d = nc.dram_tensor("out", [OUT_F, NAP], FP32, kind="ExternalOutput")

    with tile.TileContext(nc) as tc:
        with (
            tc.tile_pool(name="singles", bufs=1) as sg,
            tc.tile_pool(name="win", bufs=2) as pw,
            tc.tile_pool(name="chunk", bufs=5) as pch,
            tc.tile_pool(name="p23", bufs=1) as p23,
            tc.tile_pool(name="apsum", bufs=2, space="PSUM") as pa_ps,
            tc.tile_pool(name="spsum", bufs=2, space="PSUM") as sps,
            tc.tile_pool(name="dpsum", bufs=4, space="PSUM") as dps,
        ):
            # persistent state
            T_sb = sg.tile([128, NW, G, V], BF16, tag="T")
            S_sb = sg.tile([128, NW, 256], BF16, tag="S")
            deg_sb = sg.tile([128, NW], FP32, tag="deg")
            elocr = sg.tile([128, NW * 64], FP32, tag="elocr")
            qr = sg.tile([128, NW], FP32, tag="qr")
            aghr = sg.tile([64, 512], FP32, tag="aghr")
            w1 = sg.tile([IN_EFF, H1], FP32, tag="w1")
            w2 = sg.tile([H1, H1], FP32, tag="w2")
            w3 = sg.tile([H1, OUT_F], FP32, tag="w3")
            b1 = sg.tile([H1, 1], FP32, tag="b1")
            b2 = sg.tile([H1, 1], FP32, tag="b2")
            b3 = sg.tile([OUT_F, 1], FP32, tag="b3")
            iota = sg.tile([128, 128, CM], BF16, tag="iota")
            ident = sg.tile([128, 128], FP32, tag="ident")

            nc.sync.dma_start(out=elocr[:], in_=elocr_d[:])
            nc.sync.dma_start(out=qr[:], in_=qr_d[:])
            nc.sync.dma_start(out=b1[:], in_=b1_d[:])
            nc.sync.dma_start(out=b2[:], in_=b2_d[:])
            nc.sync.dma_start(out=b3[:], in_=b3_d[:])
            nc.sync.dma_start(out=iota[:], in_=iota_d[:])
            nc.sync.dma_start(out=aghr[:], in_=agh_d[:])
            nc.sync.dma_start(out=w1[:], in_=w1_d[:])
            nc.sync.dma_start(out=w2[:], in_=w2_d[:])
            nc.sync.dma_start(out=w3[:], in_=w3_d[:])
            nc.sync.dma_start(out=ident[:], in_=ident_d[:])

            # ---- phase A: T = E_loc @ agh (E window transposed on PE) ----
            for w in range(NW):
                et_ps = dps.tile([128, 128], FP32, tag="dps")
                nc.tensor.transpose(
                    out=et_ps[:64, :],
                    in_=elocr[:, w * 64:(w + 1) * 64],
                    identity=ident[:])
                exT = pw.tile([64, 128], FP32, tag="exT")
                nc.vector.tensor_copy(out=exT[:], in_=et_ps[:64, :])
                tp = pa_ps.tile([128, 512], FP32, tag="tps")
                nc.tensor.matmul(out=tp[:], lhsT=exT[:], rhs=aghr[:],
                                 start=True, stop=True)
                nc.vector.tensor_copy(
                    out=T_sb[:, w].rearrange("p g h -> p (g h)"), in_=tp[:])

            # ---- phases B/C/D per window ----
            for w in range(NW):
                    C = cw[w]
                    gs_t = pw.tile([128, C, 16], BF16, tag="gs")
                    gv_t = pw.tile([128, C, 48], BF16, tag="gv")
                    ix_t = pw.tile([128, C], BF16, tag="ix")
                    oh_t = pw.tile([128, 128, C], BF16, tag="oh")
                    sl = slice(woff2[w] * 128, woff2[w + 1] * 128)
                    nc.sync.dma_start(
                        out=gs_t[:],
                        in_=gs_d[sl, :].rearrange("(c p) j -> p c j", p=128))
                    nc.sync.dma_start(
                        out=gv_t[:],
                        in_=gv_d[sl, :].rearrange("(c p) j -> p c j", p=128))
                    nc.sync.dma_start(
                        out=ix_t[:],
                        in_=ix_d[sl, :].rearrange("(c p) j -> p (c j)", p=128))

                    # one-hot [p, a, c]: all operands bf16/packed -> DVE 2x
                    nc.vector.tensor_tensor(
                        out=oh_t[:],
                        in0=ix_t[:].unsqueeze(1).to_broadcast([128, 128, C]),
                        in1=iota[:, :, 0:C],
                        op=mybir.AluOpType.is_equal)

                    s_ps = sps.tile([128, 257], FP32, tag="sps")
                    for c in range(C):
                        # per-pair Gram slices prod[p,d,g,g'] = gv gv; the
                        # d-reduction happens for free on PE via three
                        # PSUM-accumulating matmuls sharing the one-hot lhsT.
                        # gs row-sum rides along as column 256 (ACT accum).
                        prod = pch.tile([128, 3, 257], BF16, tag="prod")
                        dmy = pch.tile([128, 16], BF16, tag="dmy")
                        gvv = gv_t[:, c].rearrange("p (d g) -> p d g", d=3)
                        eng = nc.vector if c % 5 == 0 else nc.gpsimd
                        eng.tensor_tensor(
                            out=prod[:, :, 0:256].rearrange(
                                "p d (g gp) -> p d g gp", g=16),
                            in0=gvv.unsqueeze(3).to_broadcast([128, 3, 16, 16]),
                            in1=gvv.unsqueeze(2).to_broadcast([128, 3, 16, 16]),
                            op=mybir.AluOpType.mult)
                        with nc.allow_low_precision("bf16 V; S accumulates in fp32 PSUM"):
                            nc.scalar.activation(
                                out=dmy[:], in_=gs_t[:, c],
                                func=mybir.ActivationFunctionType.Copy,
                                accum_out=prod[:, 0, 256:257])
                        nc.tensor.matmul(
                            out=s_ps[:], lhsT=oh_t[:, :, c], rhs=prod[:, 0],
                            start=(c == 0), stop=False)
                        nc.tensor.matmul(
                            out=s_ps[:, 0:256], lhsT=oh_t[:, :, c],
                            rhs=prod[:, 1, 0:256],
                            start=False, stop=False)
                        nc.tensor.matmul(
                            out=s_ps[:, 0:256], lhsT=oh_t[:, :, c],
                            rhs=prod[:, 2, 0:256],
                            start=False, stop=(c == C - 1))

                    nc.vector.tensor_copy(out=S_sb[:, w], in_=s_ps[:, 0:256])
                    nc.vector.tensor_copy(out=deg_sb[:, w:w + 1], in_=s_ps[:, 256:257])

                    # ---- phase C: vec[a,h] = sum_{g,g'} S T T ----
                    # p2 = T(x)T on GpSimd; p3 on DVE at 2x; (g,g') summed
                    # with a binary add-tree (2x) instead of TensorReduce (1x).
                    msg = pw.tile([128, IN_EFF], FP32, tag="msg")
                    Tw = T_sb[:, w]
                    Sw = S_sb[:, w].rearrange("p (g gp) -> p g gp", g=16)
                    p2 = p23.tile([128, V, G, G], BF16, tag="p2")
                    p3 = p23.tile([128, V, G, G], BF16, tag="p3")
                    vecb = pw.tile([128, V], BF16, tag="vecb")
                    nc.gpsimd.tensor_tensor(
                        out=p2[:],
                        in0=Tw[:].rearrange("p g h -> p h g")
                            .unsqueeze(3).to_broadcast([128, V, G, G]),
                        in1=Tw[:].rearrange("p g h -> p h g")
                            .unsqueeze(2).to_broadcast([128, V, G, G]),
                        op=mybir.AluOpType.mult)
                    nc.vector.tensor_tensor(
                        out=p3[:], in0=p2[:],
                        in1=Sw[:].unsqueeze(1).to_broadcast([128, V, G, G]),
                        op=mybir.AluOpType.mult)
                    cur = p3[:].rearrange("p h g gp -> p h (g gp)")
                    width = 256
                    tags = ["p2", "p3"]
                    ti = 0
                    while width > 2:
                        half = width // 2
                        nxt = p23.tile([128, V, half], BF16, tag=tags[ti % 2])
                        nc.vector.tensor_tensor(
                            out=nxt[:], in0=cur[:, :, 0:half],
                            in1=cur[:, :, half:width],
                            op=mybir.AluOpType.add)
                        cur = nxt[:]
                        width = half
                        ti += 1
                    nc.vector.tensor_tensor(
                        out=vecb[:].unsqueeze(2), in0=cur[:, :, 0:1],
                        in1=cur[:, :, 1:2], op=mybir.AluOpType.add)
                    nc.vector.tensor_copy(out=msg[:, 64:96], in_=vecb[:])

                    # ---- phase D: msg + MLP ----
                    nc.vector.tensor_scalar(
                        out=msg[:, 0:64], in0=elocr[:, w * 64:(w + 1) * 64],
                        scalar1=deg_sb[:, w:w + 1], scalar2=None,
                        op0=mybir.AluOpType.mult)
                    nc.vector.tensor_scalar(
                        out=msg[:, 96:97], in0=qr[:, w:w + 1],
                        scalar1=deg_sb[:, w:w + 1], scalar2=None,
                        op0=mybir.AluOpType.mult)

                    mt_ps = dps.tile([128, 128], FP32, tag="dps")
                    nc.tensor.transpose(
                        out=mt_ps[:IN_EFF, :], in_=msg[:], identity=ident[:])
                    msgT = pw.tile([IN_EFF, 128], FP32, tag="msgT")
                    nc.vector.tensor_copy(out=msgT[:], in_=mt_ps[:IN_EFF, :])

                    h1_ps = dps.tile([128, 128], FP32, tag="dps")
                    nc.tensor.matmul(out=h1_ps[:], lhsT=w1[:], rhs=msgT[:],
                                     start=True, stop=True)
                    h1 = pw.tile([H1, 128], FP32, tag="h1")
                    nc.scalar.activation(
                        out=h1[:], in_=h1_ps[:],
                        func=act_func,
                        bias=b1[:, 0:1])

                    h2_ps = dps.tile([128, 128], FP32, tag="dps")
                    nc.tensor.matmul(out=h2_ps[:], lhsT=w2[:], rhs=h1[:],
                                     start=True, stop=True)
                    h2 = pw.tile([H1, 128], FP32, tag="h2")
                    nc.scalar.activation(
                        out=h2[:], in_=h2_ps[:],
                        func=act_func,
                        bias=b2[:, 0:1])

                    o_ps = dps.tile([128, 128], FP32, tag="dps")
                    nc.tensor.matmul(out=o_ps[:OUT_F, :], lhsT=w3[:], rhs=h2[:],
                                     start=True, stop=True)
                    o_sb = pw.tile([OUT_F, 128], FP32, tag="osb")
                    nc.vector.tensor_scalar(
                        out=o_sb[:], in0=o_ps[:OUT_F, :],
                        scalar1=b3[:, 0:1], scalar2=None,
                        op0=mybir.AluOpType.add)
                    nc.sync.dma_start(
                        out=out_d[:, w * 128:(w + 1) * 128], in_=o_sb[:])
    if not nc.is_finalized():
        nc.finalize()
    return nc


def kernel(**inputs):
    cores, cw = _host_prep(inputs)
    E = np.asarray(inputs["atomic_embedding"], np.float32)
    q = np.asarray(inputs["partial_charges"], np.float32)[:, 0]
    agh_rs = np.ascontiguousarray(
        np.asarray(inputs["agh"], np.float32).reshape(64, 512))
    W1e = np.ascontiguousarray(np.asarray(inputs["W1"], np.float32)[:IN_EFF])
    W2 = np.asarray(inputs["W2"], np.float32)
    W3 = np.asarray(inputs["W3"], np.float32)
    b1 = np.ascontiguousarray(np.asarray(inputs["b1"], np.float32)[:, None])
    b2 = np.ascontiguousarray(np.asarray(inputs["b2"], np.float32)[:, None])
    b3 = np.ascontiguousarray(np.asarray(inputs["b3"], np.float32)[:, None])
    iota = np.broadcast_to(
        np.arange(128, dtype=np.float32)[None, :, None], (128, 128, max(cw))
    ).astype(ml_dtypes.bfloat16)
    ident = np.eye(128, dtype=np.float32)

    in_maps = []
    for c in range(N_CORES):
        gs_p, gv_p, ix_p = cores[c]
        E_loc = np.zeros((NAP, 64), np.float32)
        E_loc[:NA] = E[c * NA:(c + 1) * NA]
        q_loc = np.zeros((NAP,), np.float32)
        q_loc[:NA] = q[c * NA:(c + 1) * NA]
        elocr = np.ascontiguousarray(
            E_loc.reshape(NW, 128, 64).transpose(1, 0, 2).reshape(128, NW * 64))
        qr = np.ascontiguousarray(q_loc.reshape(NW, 128).T)
        in_maps.append({
            "gsp": gs_p, "gvp": gv_p, "ixp": ix_p,
            "elocr": elocr, "qr": qr, "aghr": agh_rs,
            "w1e": W1e, "w2": W2, "w3": W3,
            "b1c": b1, "b2c": b2, "b3c": b3,
            "iota": iota, "ident": ident,
        })

    nc = _build_nc(cw)
    res = run_bass_kernel_spmd(nc, in_maps, list(range(N_CORES)))
    if getattr(res, "exec_time_ns", None):
        print(f"HW exec time: {res.exec_time_ns} ns")
    outs = [res.results[c]["out"][:, :NA] for c in range(N_CORES)]
    full = np.concatenate(outs, axis=1).T.astype(np.float32)  # [50000, 66]
    delta_q = np.ascontiguousarray(full[:, 0:1])
    f = np.ascontiguousarray(full[:, 1:2])
    delta_a = np.ascontiguousarray(full[:, 2:])
    return delta_a, delta_q, f


# revision 24
# speedup vs baseline: 1.0732x; 1.0732x over previous
"""Trainium2 Bass kernel for AimNet2Core message-passing block.

Strategy:
  - a_j = E[idx_j] depends only on the destination atom, so the radial branch
    collapses to  radial_emb[n] = E[n] * deg[n],  radial_q[n] = q[n] * deg[n]
    with deg[n] = segment_sum(gs.sum(-1)).
  - The vector branch uses the Gram identity
        vec_emb[n,h] = sum_{g,g'} S[n,g,g'] T[n,g,h] T[n,g',h]
    with S[n] = segment_sum_p(gv[p]^T gv[p])  (16x16 per pair, summed over d=3)
    and T = E @ agh  computed per atom (not per pair).
  - Host shards pairs by owner atom (sort by idx_j): core c owns atoms
    [6250c, 6250(c+1)).  Within a core, atoms are tiled into 49 windows of
    128; each window's pairs are processed in chunks of 128 and segment-summed
    via a one-hot matmul accumulated in PSUM.  No collectives needed.
  - MLP is computed per window entirely on-chip; output is [66, atoms].
"""
import math
import numpy as np
import ml_dtypes

import concourse.bass as bass
import concourse.bacc as bacc
import concourse.mybir as mybir
import concourse.tile as tile
from concourse.bass_utils import run_bass_kernel_spmd

N_ATOMS = 50000
N_CORES = 8
NA = N_ATOMS // N_CORES      # 6250
NW = 49                      # atom windows per core
NAP = NW * 128               # 6272 padded atoms per core
G = 16
V = 32
F = 64
H1 = 128
OUT_F = 66
IN_EFF = 97                  # 64 + 32 + 1 effective msg features (vec_q==0)

FP32 = mybir.dt.float32
BF16 = mybir.dt.bfloat16


def _host_prep(inputs):
    idx = np.asarray(inputs["pair_indices"][1], dtype=np.int64)
    P = idx.shape[0]
    order = np.argsort(idx, kind="stable")
    sidx = idx[order]
    gs_s = np.ascontiguousarray(np.asarray(inputs["gs"], np.float32)[order])
    gv_s = np.ascontiguousarray(
        np.asarray(inputs["gv"], np.float32)[order].reshape(P, 48))
    bounds = np.searchsorted(sidx, [NA * c for c in range(N_CORES + 1)])

    percore = []
    cw = np.ones(NW, np.int64)
    for c in range(N_CORES):
        lo, hi = bounds[c], bounds[c + 1]
        lidx = (sidx[lo:hi] - NA * c).astype(np.int64)
        w = lidx >> 7
        cnt = np.bincount(w, minlength=NW)
        cw = np.maximum(cw, (cnt + 127) // 128)
        percore.append((lo, hi, lidx, w, cnt))

    woff2 = np.concatenate([[0], np.cumsum(cw)])  # chunk offsets per window
    nslot = int(woff2[-1]) * 128
    cores = []
    for c in range(N_CORES):
        lo, hi, lidx, w, cnt = percore[c]
        gs_p = np.zeros((nslot, 16), ml_dtypes.bfloat16)
        gv_p = np.zeros((nslot, 48), ml_dtypes.bfloat16)
        ix_p = np.zeros((nslot, 1), ml_dtypes.bfloat16)
        woff = np.concatenate([[0], np.cumsum(cnt)[:-1]])
        pos_in_w = np.arange(len(lidx)) - woff[w]
        slot = woff2[w] * 128 + pos_in_w
        gs_p[slot] = gs_s[lo:hi]
        gv_p[slot] = gv_s[lo:hi]
        ix_p[slot, 0] = (lidx & 127).astype(ml_dtypes.bfloat16)
        cores.append((gs_p, gv_p, ix_p))
    return cores, [int(x) for x in cw]


def _build_nc(cw, sim_safe=False):
    # sim_safe: CoreSim doesn't implement Gelu; Relu is timing-identical
    # for cost-model profiling runs.
    act_func = (mybir.ActivationFunctionType.Relu if sim_safe
                else mybir.ActivationFunctionType.Gelu)
    nc = bacc.Bacc()
    CM = max(cw)
    woff2 = [0]
    for x in cw:
        woff2.append(woff2[-1] + x)
    nslot = woff2[-1] * 128

    gs_d = nc.dram_tensor("gsp", [nslot, 16], BF16, kind="ExternalInput")
    gv_d = nc.dram_tensor("gvp", [nslot, 48], BF16, kind="ExternalInput")
    ix_d = nc.dram_tensor("ixp", [nslot, 1], BF16, kind="ExternalInput")
    elocr_d = nc.dram_tensor("elocr", [128, NW * 64], FP32, kind="ExternalInput")
    qr_d = nc.dram_tensor("qr", [128, NW], FP32, kind="ExternalInput")
    agh_d = nc.dram_tensor("aghr", [64, 512], FP32, kind="ExternalInput")
    w1_d = nc.dram_tensor("w1e", [IN_EFF, H1], FP32, kind="ExternalInput")
    w2_d = nc.dram_tensor("w2", [H1, H1], FP32, kind="ExternalInput")
    w3_d = nc.dram_tensor("w3", [H1, OUT_F], FP32, kind="ExternalInput")
    b1_d = nc.dram_tensor("b1c", [H1, 1], FP32, kind="ExternalInput")
    b2_d = nc.dram_tensor("b2c", [H1, 1], FP32, kind="ExternalInput")
    b3_d = nc.dram_tensor("b3c", [OUT_F, 1], FP32, kind="ExternalInput")
    iota_d = nc.dram_tensor("iota", [128, 128, CM], BF16, kind="ExternalInput")
    ident_d = nc.dram_tensor("ident", [128, 128], FP32, kind="ExternalInput")
    out_d = nc.dram_tensor("out", [OUT_F, NAP], FP32, kind="ExternalOutput")

    with tile.TileContext(nc) as tc:
        with (
            tc.tile_pool(name="singles", bufs=1) as sg,
            tc.tile_pool(name="win", bufs=2) as pw,
            tc.tile_pool(name="chunk", bufs=5) as pch,
            tc.tile_pool(name="p23", bufs=1) as p23,
            tc.tile_pool(name="apsum", bufs=1, space="PSUM") as pa_ps,
            tc.tile_pool(name="spsum", bufs=3, space="PSUM") as sps,
            tc.tile_pool(name="dpsum", bufs=4, space="PSUM") as dps,
        ):
            # persistent state
            T_sb = sg.tile([128, NW, G, V], BF16, tag="T")
            S_sb = sg.tile([128, NW, 256], BF16, tag="S")
            deg_sb = sg.tile([128, NW], FP32, tag="deg")
            elocr = sg.tile([128, NW * 64], FP32, tag="elocr")
            qr = sg.tile([128, NW], FP32, tag="qr")
            aghr = sg.tile([64, 512], FP32, tag="aghr")
            w1 = sg.tile([IN_EFF, H1], FP32, tag="w1")
            w2 = sg.tile([H1, H1], FP32, tag="w2")
            w3 = sg.tile([H1, OUT_F], FP32, tag="w3")
            b1 = sg.tile([H1, 1], FP32, tag="b1")
            b2 = sg.tile([H1, 1], FP32, tag="b2")
            b3 = sg.tile([OUT_F, 1], FP32, tag="b3")
            iota = sg.tile([128, 128, CM], BF16, tag="iota")
            ident = sg.tile([128, 128], FP32, tag="ident")

            nc.sync.dma_start(out=elocr[:], in_=elocr_d[:])
            nc.sync.dma_start(out=qr[:], in_=qr_d[:])
            nc.sync.dma_start(out=b1[:], in_=b1_d[:])
            nc.sync.dma_start(out=b2[:], in_=b2_d[:])
            nc.sync.dma_start(out=b3[:], in_=b3_d[:])
            nc.sync.dma_start(out=iota[:], in_=iota_d[:])
            nc.sync.dma_start(out=aghr[:], in_=agh_d[:])
            nc.sync.dma_start(out=w1[:], in_=w1_d[:])
            nc.sync.dma_start(out=w2[:], in_=w2_d[:])
            nc.sync.dma_start(out=w3[:], in_=w3_d[:])
            nc.sync.dma_start(out=ident[:], in_=ident_d[:])

            # ---- phase A: T = E_loc @ agh (E window transposed on PE) ----
            for w in range(NW):
                et_ps = dps.tile([128, 128], FP32, tag="dps")
                nc.tensor.transpose(
                    out=et_ps[:64, :],
                    in_=elocr[:, w * 64:(w + 1) * 64],
                    identity=ident[:])
                exT = pw.tile([64, 128], FP32, tag="exT")
                nc.vector.tensor_copy(out=exT[:], in_=et_ps[:64, :])
                tp = pa_ps.tile([128, 512], FP32, tag="tps")
                nc.tensor.matmul(out=tp[:], lhsT=exT[:], rhs=aghr[:],
                                 start=True, stop=True)
                nc.vector.tensor_copy(
                    out=T_sb[:, w].rearrange("p g h -> p (g h)"), in_=tp[:])

            # ---- phases B/C/D per window ----
            for w in range(NW):
                    C = cw[w]
                    gs_t = pw.tile([128, C, 16], BF16, tag="gs")
                    gv_t = pw.tile([128, C, 48], BF16, tag="gv")
                    ix_t = pw.tile([128, C], BF16, tag="ix")
                    oh_t = pw.tile([128, 128, C], BF16, tag="oh")
                    sl = slice(woff2[w] * 128, woff2[w + 1] * 128)
                    nc.sync.dma_start(
                        out=gs_t[:],
                        in_=gs_d[sl, :].rearrange("(c p) j -> p c j", p=128))
                    nc.sync.dma_start(
                        out=gv_t[:],
                        in_=gv_d[sl, :].rearrange("(c p) j -> p c j", p=128))
                    nc.sync.dma_start(
                        out=ix_t[:],
                        in_=ix_d[sl, :].rearrange("(c p) j -> p (c j)", p=128))

                    # one-hot [p, a, c]: all operands bf16/packed -> DVE 2x
                    nc.vector.tensor_tensor(
                        out=oh_t[:],
                        in0=ix_t[:].unsqueeze(1).to_broadcast([128, 128, C]),
                        in1=iota[:, :, 0:C],
                        op=mybir.AluOpType.is_equal)

                    s_ps = sps.tile([128, 257], FP32, tag="sps")
                    for c in range(C):
                        # per-pair Gram slices prod[p,d,g,g'] = gv gv; the
                        # d-reduction happens for free on PE via three
                        # PSUM-accumulating matmuls sharing the one-hot lhsT.
                        # gs row-sum rides along as column 256 (ACT accum).
                        prod = pch.tile([128, 3, 257], BF16, tag="prod")
                        dmy = pch.tile([128, 16], BF16, tag="dmy")
                        gvv = gv_t[:, c].rearrange("p (d g) -> p d g", d=3)
                        eng = nc.vector if c % 5 == 0 else nc.gpsimd
                        eng.tensor_tensor(
                            out=prod[:, :, 0:256].rearrange(
                                "p d (g gp) -> p d g gp", g=16),
                            in0=gvv.unsqueeze(3).to_broadcast([128, 3, 16, 16]),
                            in1=gvv.unsqueeze(2).to_broadcast([128, 3, 16, 16]),
                            op=mybir.AluOpType.mult)
                        with nc.allow_low_precision("bf16 V; S accumulates in fp32 PSUM"):
                            nc.scalar.activation(
                                out=dmy[:], in_=gs_t[:, c],
                                func=mybir.ActivationFunctionType.Copy,
                                accum_out=prod[:, 0, 256:257])
                        nc.tensor.matmul(
                            out=s_ps[:], lhsT=oh_t[:, :, c], rhs=prod[:, 0],
                            start=(c == 0), stop=False)
                        nc.tensor.matmul(
                            out=s_ps[:, 0:256], lhsT=oh_t[:, :, c],
                            rhs=prod[:, 1, 0:256],
                            start=False, stop=False)
                        nc.tensor.matmul(
                            out=s_ps[:, 0:256], lhsT=oh_t[:, :, c],
                            rhs=prod[:, 2, 0:256],
                            start=False, stop=(c == C - 1))

                    nc.vector.tensor_copy(out=S_sb[:, w], in_=s_ps[:, 0:256])
                    nc.vector.tensor_copy(out=deg_sb[:, w:w + 1], in_=s_ps[:, 256:257])

                    # ---- phase C: vec[a,h] = sum_{g,g'} S T T ----
                    # p2 = T(x)T on GpSimd; p3 on DVE at 2x; (g,g') summed
                    # with a binary add-tree (2x) instead of TensorReduce (1x).
                    msg = pw.tile([128, IN_EFF], FP32, tag="msg")
                    Tw = T_sb[:, w]
                    Sw = S_sb[:, w].rearrange("p (g gp) -> p g gp", g=16)
                    p2 = p23.tile([128, V, G, G], BF16, tag="p2")
                    p3 = p23.tile([128, V, G, G], BF16, tag="p3")
                    vecb = pw.tile([128, V], BF16, tag="vecb")
                    nc.gpsimd.tensor_tensor(
                        out=p2[:],
                        in0=Tw[:].rearrange("p g h -> p h g")
                            .unsqueeze(3).to_broadcast([128, V, G, G]),
                        in1=Tw[:].rearrange("p g h -> p h g")
                            .unsqueeze(2).to_broadcast([128, V, G, G]),
                        op=mybir.AluOpType.mult)
                    nc.vector.tensor_tensor(
                        out=p3[:], in0=p2[:],
                        in1=Sw[:].unsqueeze(1).to_broadcast([128, V, G, G]),
                        op=mybir.AluOpType.mult)
                    cur = p3[:].rearrange("p h g gp -> p h (g gp)")
                    width = 256
                    tags = ["ta", "tb"]
                    ti = 0
                    while width > 2:
                        half = width // 2
                        nxt = p23.tile([128, V, half], BF16, tag=tags[ti % 2])
                        nc.vector.tensor_tensor(
                            out=nxt[:], in0=cur[:, :, 0:half],
                            in1=cur[:, :, half:width],
                            op=mybir.AluOpType.add)
                        cur = nxt[:]
                        width = half
                        ti += 1
                    nc.vector.tensor_tensor(
                        out=vecb[:].unsqueeze(2), in0=cur[:, :, 0:1],
                        in1=cur[:, :, 1:2], op=mybir.AluOpType.add)
                    nc.vector.tensor_copy(out=msg[:, 64:96], in_=vecb[:])

                    # ---- phase D: msg + MLP ----
                    nc.vector.tensor_scalar(
                        out=msg[:, 0:64], in0=elocr[:, w * 64:(w + 1) * 64],
                        scalar1=deg_sb[:, w:w + 1], scalar2=None,
                        op0=mybir.AluOpType.mult)
                    nc.vector.tensor_scalar(
                        out=msg[:, 96:97], in0=qr[:, w:w + 1],
                        scalar1=deg_sb[:, w:w + 1], scalar2=None,
                        op0=mybir.AluOpType.mult)

                    mt_ps = dps.tile([128, 128], FP32, tag="dps")
                    nc.tensor.transpose(
                        out=mt_ps[:IN_EFF, :], in_=msg[:], identity=ident[:])
                    msgT = pw.tile([IN_EFF, 128], FP32, tag="msgT")
                    nc.vector.tensor_copy(out=msgT[:], in_=mt_ps[:IN_EFF, :])

                    h1_ps = dps.tile([128, 128], FP32, tag="dps")
                    nc.tensor.matmul(out=h1_ps[:], lhsT=w1[:], rhs=msgT[:],
                                     start=True, stop=True)
                    h1 = pw.tile([H1, 128], FP32, tag="h1")
                    nc.scalar.activation(
                        out=h1[:], in_=h1_ps[:],
                        func=act_func,
                        bias=b1[:, 0:1])

                    h2_ps = dps.tile([128, 128], FP32, tag="dps")
                    nc.tensor.matmul(out=h2_ps[:], lhsT=w2[:], rhs=h1[:],
                                     start=True, stop=True)
                    h2 = pw.tile([H1, 128], FP32, tag="h2")
                    nc.scalar.activation(
                        out=h2[:], in_=h2_ps[:],
                        func=act_func,
                        bias=b2[:, 0:1])

                    o_ps = dps.tile([128, 128], FP32, tag="dps")
                    nc.tensor.matmul(out=o_ps[:OUT_F, :], lhsT=w3[:], rhs=h2[:],
                                     start=True, stop=True)
                    o_sb = pw.tile([OUT_F, 128], FP32, tag="osb")
                    nc.vector.tensor_scalar(
                        out=o_sb[:], in0=o_ps[:OUT_F, :],
                        scalar1=b3[:, 0:1], scalar2=None,
                        op0=mybir.AluOpType.add)
                    nc.sync.dma_start(
                        out=out_d[:, w * 128:(w + 1) * 128], in_=o_sb[:])
    if not nc.is_finalized():
        nc.finalize()
    return nc


def kernel(**inputs):
    cores, cw = _host_prep(inputs)
    E = np.asarray(inputs["atomic_embedding"], np.float32)
    q = np.asarray(inputs["partial_charges"], np.float32)[:, 0]
    agh_rs = np.ascontiguousarray(
        np.asarray(inputs["agh"], np.float32).reshape(64, 512))
    W1e = np.ascontiguousarray(np.asarray(inputs["W1"], np.float32)[:IN_EFF])
    W2 = np.asarray(inputs["W2"], np.float32)
    W3 = np.asarray(inputs["W3"], np.float32)
    b1 = np.ascontiguousarray(np.asarray(inputs["b1"], np.float32)[:, None])
    b2 = np.ascontiguousarray(np.asarray(inputs["b2"], np.float32)[:, None])
    b3 = np.ascontiguousarray(np.asarray(inputs["b3"], np.float32)[:, None])
    iota = np.broadcast_to(
        np.arange(128, dtype=np.float32)[None, :, None], (128, 128, max(cw))
    ).astype(ml_dtypes.bfloat16)
    ident = np.eye(128, dtype=np.float32)

    in_maps = []
    for c in range(N_CORES):
        gs_p, gv_p, ix_p = cores[c]
        E_loc = np.zeros((NAP, 64), np.float32)
        E_loc[:NA] = E[c * NA:(c + 1) * NA]
        q_loc = np.zeros((NAP,), np.float32)
        q_loc[:NA] = q[c * NA:(c + 1) * NA]
        elocr = np.ascontiguousarray(
            E_loc.reshape(NW, 128, 64).transpose(1, 0, 2).reshape(128, NW * 64))
        qr = np.ascontiguousarray(q_loc.reshape(NW, 128).T)
        in_maps.append({
            "gsp": gs_p, "gvp": gv_p, "ixp": ix_p,
            "elocr": elocr, "qr": qr, "aghr": agh_rs,
            "w1e": W1e, "w2": W2, "w3": W3,
            "b1c": b1, "b2c": b2, "b3c": b3,
            "iota": iota, "ident": ident,
        })

    nc = _build_nc(cw)
    res = run_bass_kernel_spmd(nc, in_maps, list(range(N_CORES)))
    if getattr(res, "exec_time_ns", None):
        print(f"HW exec time: {res.exec_time_ns} ns")
    outs = [res.results[c]["out"][:, :NA] for c in range(N_CORES)]
    full = np.concatenate(outs, axis=1).T.astype(np.float32)  # [50000, 66]
    delta_q = np.ascontiguousarray(full[:, 0:1])
    f = np.ascontiguousarray(full[:, 1:2])
    delta_a = np.ascontiguousarray(full[:, 2:])
    return delta_a, delta_q, f


# revision 25
# speedup vs baseline: 1.1395x; 1.0618x over previous
"""Trainium2 Bass kernel for AimNet2Core message-passing block.

Strategy:
  - a_j = E[idx_j] depends only on the destination atom, so the radial branch
    collapses to  radial_emb[n] = E[n] * deg[n],  radial_q[n] = q[n] * deg[n]
    with deg[n] = segment_sum(gs.sum(-1)).
  - The vector branch uses the Gram identity
        vec_emb[n,h] = sum_{g,g'} S[n,g,g'] T[n,g,h] T[n,g',h]
    with S[n] = segment_sum_p(gv[p]^T gv[p])  (16x16 per pair, summed over d=3)
    and T = E @ agh  computed per atom (not per pair).
  - Host shards pairs by owner atom (sort by idx_j): core c owns atoms
    [6250c, 6250(c+1)).  Within a core, atoms are tiled into 49 windows of
    128; each window's pairs are processed in chunks of 128 and segment-summed
    via a one-hot matmul accumulated in PSUM.  No collectives needed.
  - MLP is computed per window entirely on-chip; output is [66, atoms].
"""
import math
import numpy as np
import ml_dtypes

import concourse.bass as bass
import concourse.bacc as bacc
import concourse.mybir as mybir
import concourse.tile as tile
from concourse.bass_utils import run_bass_kernel_spmd

N_ATOMS = 50000
N_CORES = 8
NA = N_ATOMS // N_CORES      # 6250
NW = 49                      # atom windows per core
NAP = NW * 128               # 6272 padded atoms per core
G = 16
V = 32
F = 64
H1 = 128
OUT_F = 66
IN_EFF = 97                  # 64 + 32 + 1 effective msg features (vec_q==0)

FP32 = mybir.dt.float32
BF16 = mybir.dt.bfloat16


def _host_prep(inputs):
    idx = np.asarray(inputs["pair_indices"][1], dtype=np.int64)
    P = idx.shape[0]
    order = np.argsort(idx, kind="stable")
    sidx = idx[order]
    gs_s = np.ascontiguousarray(np.asarray(inputs["gs"], np.float32)[order])
    gv_s = np.ascontiguousarray(
        np.asarray(inputs["gv"], np.float32)[order].reshape(P, 48))
    bounds = np.searchsorted(sidx, [NA * c for c in range(N_CORES + 1)])

    percore = []
    cw = np.ones(NW, np.int64)
    for c in range(N_CORES):
        lo, hi = bounds[c], bounds[c + 1]
        lidx = (sidx[lo:hi] - NA * c).astype(np.int64)
        w = lidx >> 7
        cnt = np.bincount(w, minlength=NW)
        cw = np.maximum(cw, (cnt + 127) // 128)
        percore.append((lo, hi, lidx, w, cnt))

    woff2 = np.concatenate([[0], np.cumsum(cw)])  # chunk offsets per window
    nslot = int(woff2[-1]) * 128
    cores = []
    for c in range(N_CORES):
        lo, hi, lidx, w, cnt = percore[c]
        gs_p = np.zeros((nslot, 16), ml_dtypes.bfloat16)
        gv_p = np.zeros((nslot, 48), ml_dtypes.bfloat16)
        ix_p = np.zeros((nslot, 1), ml_dtypes.bfloat16)
        woff = np.concatenate([[0], np.cumsum(cnt)[:-1]])
        pos_in_w = np.arange(len(lidx)) - woff[w]
        slot = woff2[w] * 128 + pos_in_w
        gs_p[slot] = gs_s[lo:hi]
        gv_p[slot] = gv_s[lo:hi]
        ix_p[slot, 0] = (lidx & 127).astype(ml_dtypes.bfloat16)
        cores.append((gs_p, gv_p, ix_p))
    return cores, [int(x) for x in cw]


def _build_nc(cw, sim_safe=False):
    # sim_safe: CoreSim doesn't implement Gelu; Relu is timing-identical
    # for cost-model profiling runs.
    act_func = (mybir.ActivationFunctionType.Relu if sim_safe
                else mybir.ActivationFunctionType.Gelu)
    nc = bacc.Bacc()
    CM = max(cw)
    woff2 = [0]
    for x in cw:
        woff2.append(woff2[-1] + x)
    nslot = woff2[-1] * 128

    gs_d = nc.dram_tensor("gsp", [nslot, 16], BF16, kind="ExternalInput")
    gv_d = nc.dram_tensor("gvp", [nslot, 48], BF16, kind="ExternalInput")
    ix_d = nc.dram_tensor("ixp", [nslot, 1], BF16, kind="ExternalInput")
    elocr_d = nc.dram_tensor("elocr", [128, NW * 64], FP32, kind="ExternalInput")
    qr_d = nc.dram_tensor("qr", [128, NW], FP32, kind="ExternalInput")
    agh_d = nc.dram_tensor("aghr", [64, 512], FP32, kind="ExternalInput")
    w1_d = nc.dram_tensor("w1e", [IN_EFF, H1], FP32, kind="ExternalInput")
    w2_d = nc.dram_tensor("w2", [H1, H1], FP32, kind="ExternalInput")
    w3_d = nc.dram_tensor("w3", [H1, OUT_F], FP32, kind="ExternalInput")
    b1_d = nc.dram_tensor("b1c", [H1, 1], FP32, kind="ExternalInput")
    b2_d = nc.dram_tensor("b2c", [H1, 1], FP32, kind="ExternalInput")
    b3_d = nc.dram_tensor("b3c", [OUT_F, 1], FP32, kind="ExternalInput")
    iota_d = nc.dram_tensor("iota", [128, 128, CM], BF16, kind="ExternalInput")
    ident_d = nc.dram_tensor("ident", [128, 128], FP32, kind="ExternalInput")
    out_d = nc.dram_tensor("out", [OUT_F, NAP], FP32, kind="ExternalOutput")

    with tile.TileContext(nc) as tc:
        with (
            tc.tile_pool(name="singles", bufs=1) as sg,
            tc.tile_pool(name="win", bufs=2) as pw,
            tc.tile_pool(name="chunk", bufs=7) as pch,
            tc.tile_pool(name="p23", bufs=1) as p23,
            tc.tile_pool(name="apsum", bufs=1, space="PSUM") as pa_ps,
            tc.tile_pool(name="spsum", bufs=3, space="PSUM") as sps,
            tc.tile_pool(name="dpsum", bufs=4, space="PSUM") as dps,
        ):
            # persistent state
            T_sb = sg.tile([128, NW, G, V], BF16, tag="T")
            S_sb = sg.tile([128, NW, 256], BF16, tag="S")
            deg_sb = sg.tile([128, NW], FP32, tag="deg")
            elocr = sg.tile([128, NW * 64], FP32, tag="elocr")
            qr = sg.tile([128, NW], FP32, tag="qr")
            aghr = sg.tile([64, 512], FP32, tag="aghr")
            w1 = sg.tile([IN_EFF, H1], FP32, tag="w1")
            w2 = sg.tile([H1, H1], FP32, tag="w2")
            w3 = sg.tile([H1, OUT_F], FP32, tag="w3")
            b1 = sg.tile([H1, 1], FP32, tag="b1")
            b2 = sg.tile([H1, 1], FP32, tag="b2")
            b3 = sg.tile([OUT_F, 1], FP32, tag="b3")
            iota = sg.tile([128, 128, CM], BF16, tag="iota")
            ident = sg.tile([128, 128], FP32, tag="ident")

            nc.sync.dma_start(out=elocr[:], in_=elocr_d[:])
            nc.sync.dma_start(out=qr[:], in_=qr_d[:])
            nc.sync.dma_start(out=b1[:], in_=b1_d[:])
            nc.sync.dma_start(out=b2[:], in_=b2_d[:])
            nc.sync.dma_start(out=b3[:], in_=b3_d[:])
            nc.sync.dma_start(out=iota[:], in_=iota_d[:])
            nc.sync.dma_start(out=aghr[:], in_=agh_d[:])
            nc.sync.dma_start(out=w1[:], in_=w1_d[:])
            nc.sync.dma_start(out=w2[:], in_=w2_d[:])
            nc.sync.dma_start(out=w3[:], in_=w3_d[:])
            nc.sync.dma_start(out=ident[:], in_=ident_d[:])

            # ---- phase A: T = E_loc @ agh (E window transposed on PE) ----
            for w in range(NW):
                et_ps = dps.tile([128, 128], FP32, tag="dps")
                nc.tensor.transpose(
                    out=et_ps[:64, :],
                    in_=elocr[:, w * 64:(w + 1) * 64],
                    identity=ident[:])
                exT = pw.tile([64, 128], FP32, tag="exT")
                nc.vector.tensor_copy(out=exT[:], in_=et_ps[:64, :])
                tp = pa_ps.tile([128, 512], FP32, tag="tps")
                nc.tensor.matmul(out=tp[:], lhsT=exT[:], rhs=aghr[:],
                                 start=True, stop=True)
                nc.vector.tensor_copy(
                    out=T_sb[:, w].rearrange("p g h -> p (g h)"), in_=tp[:])

            # ---- phases B/C/D per window ----
            for w in range(NW):
                    C = cw[w]
                    gs_t = pw.tile([128, C, 16], BF16, tag="gs")
                    gv_t = pw.tile([128, C, 48], BF16, tag="gv")
                    ix_t = pw.tile([128, C], BF16, tag="ix")
                    oh_t = pw.tile([128, 128, C], BF16, tag="oh")
                    sl = slice(woff2[w] * 128, woff2[w + 1] * 128)
                    nc.sync.dma_start(
                        out=gs_t[:],
                        in_=gs_d[sl, :].rearrange("(c p) j -> p c j", p=128))
                    nc.sync.dma_start(
                        out=gv_t[:],
                        in_=gv_d[sl, :].rearrange("(c p) j -> p c j", p=128))
                    nc.sync.dma_start(
                        out=ix_t[:],
                        in_=ix_d[sl, :].rearrange("(c p) j -> p (c j)", p=128))

                    # one-hot [p, a, c]: all operands bf16/packed -> DVE 2x
                    nc.vector.tensor_tensor(
                        out=oh_t[:],
                        in0=ix_t[:].unsqueeze(1).to_broadcast([128, 128, C]),
                        in1=iota[:, :, 0:C],
                        op=mybir.AluOpType.is_equal)

                    s_ps = sps.tile([128, 257], FP32, tag="sps")
                    for c in range(C):
                        # per-pair Gram slices prod[p,d,g,g'] = gv gv; the
                        # d-reduction happens for free on PE via three
                        # PSUM-accumulating matmuls sharing the one-hot lhsT.
                        # gs row-sum rides along as column 256 (ACT accum).
                        prod = pch.tile([128, 3, 257], BF16, tag="prod")
                        dmy = pch.tile([128, 16], BF16, tag="dmy")
                        gvv = gv_t[:, c].rearrange("p (d g) -> p d g", d=3)
                        eng = nc.vector if c % 6 == 0 else nc.gpsimd
                        eng.tensor_tensor(
                            out=prod[:, :, 0:256].rearrange(
                                "p d (g gp) -> p d g gp", g=16),
                            in0=gvv.unsqueeze(3).to_broadcast([128, 3, 16, 16]),
                            in1=gvv.unsqueeze(2).to_broadcast([128, 3, 16, 16]),
                            op=mybir.AluOpType.mult)
                        with nc.allow_low_precision("bf16 V; S accumulates in fp32 PSUM"):
                            nc.scalar.activation(
                                out=dmy[:], in_=gs_t[:, c],
                                func=mybir.ActivationFunctionType.Copy,
                                accum_out=prod[:, 0, 256:257])
                        nc.tensor.matmul(
                            out=s_ps[:], lhsT=oh_t[:, :, c], rhs=prod[:, 0],
                            start=(c == 0), stop=False)
                        nc.tensor.matmul(
                            out=s_ps[:, 0:256], lhsT=oh_t[:, :, c],
                            rhs=prod[:, 1, 0:256],
                            start=False, stop=False)
                        nc.tensor.matmul(
                            out=s_ps[:, 0:256], lhsT=oh_t[:, :, c],
                            rhs=prod[:, 2, 0:256],
                            start=False, stop=(c == C - 1))

                    nc.vector.tensor_copy(out=S_sb[:, w], in_=s_ps[:, 0:256])
                    nc.vector.tensor_copy(out=deg_sb[:, w:w + 1], in_=s_ps[:, 256:257])

                    # ---- phase C: vec[a,h] = sum_{g,g'} S T T ----
                    # p2 = T(x)T on GpSimd; p3 on DVE at 2x; (g,g') summed
                    # with a binary add-tree (2x) instead of TensorReduce (1x).
                    msg = pw.tile([128, IN_EFF], FP32, tag="msg")
                    Tw = T_sb[:, w]
                    Sw = S_sb[:, w].rearrange("p (g gp) -> p g gp", g=16)
                    p2 = p23.tile([128, V, G, G], BF16, tag="p2")
                    p3 = p23.tile([128, V, G, G], BF16, tag="p3")
                    vecb = pw.tile([128, V], BF16, tag="vecb")
                    nc.gpsimd.tensor_tensor(
                        out=p2[:],
                        in0=Tw[:].rearrange("p g h -> p h g")
                            .unsqueeze(3).to_broadcast([128, V, G, G]),
                        in1=Tw[:].rearrange("p g h -> p h g")
                            .unsqueeze(2).to_broadcast([128, V, G, G]),
                        op=mybir.AluOpType.mult)
                    nc.vector.tensor_tensor(
                        out=p3[:], in0=p2[:],
                        in1=Sw[:].unsqueeze(1).to_broadcast([128, V, G, G]),
                        op=mybir.AluOpType.mult)
                    cur = p3[:].rearrange("p h g gp -> p h (g gp)")
                    width = 256
                    tags = ["ta", "tb"]
                    ti = 0
                    while width > 2:
                        half = width // 2
                        nxt = p23.tile([128, V, half], BF16, tag=tags[ti % 2])
                        nc.vector.tensor_tensor(
                            out=nxt[:], in0=cur[:, :, 0:half],
                            in1=cur[:, :, half:width],
                            op=mybir.AluOpType.add)
                        cur = nxt[:]
                        width = half
                        ti += 1
                    nc.vector.tensor_tensor(
                        out=vecb[:].unsqueeze(2), in0=cur[:, :, 0:1],
                        in1=cur[:, :, 1:2], op=mybir.AluOpType.add)
                    nc.vector.tensor_copy(out=msg[:, 64:96], in_=vecb[:])

                    # ---- phase D: msg + MLP ----
                    nc.vector.tensor_scalar(
                        out=msg[:, 0:64], in0=elocr[:, w * 64:(w + 1) * 64],
                        scalar1=deg_sb[:, w:w + 1], scalar2=None,
                        op0=mybir.AluOpType.mult)
                    nc.vector.tensor_scalar(
                        out=msg[:, 96:97], in0=qr[:, w:w + 1],
                        scalar1=deg_sb[:, w:w + 1], scalar2=None,
                        op0=mybir.AluOpType.mult)

                    mt_ps = dps.tile([128, 128], FP32, tag="dps")
                    nc.tensor.transpose(
                        out=mt_ps[:IN_EFF, :], in_=msg[:], identity=ident[:])
                    msgT = pw.tile([IN_EFF, 128], FP32, tag="msgT")
                    nc.vector.tensor_copy(out=msgT[:], in_=mt_ps[:IN_EFF, :])

                    h1_ps = dps.tile([128, 128], FP32, tag="dps")
                    nc.tensor.matmul(out=h1_ps[:], lhsT=w1[:], rhs=msgT[:],
                                     start=True, stop=True)
                    h1 = pw.tile([H1, 128], FP32, tag="h1")
                    nc.scalar.activation(
                        out=h1[:], in_=h1_ps[:],
                        func=act_func,
                        bias=b1[:, 0:1])

                    h2_ps = dps.tile([128, 128], FP32, tag="dps")
                    nc.tensor.matmul(out=h2_ps[:], lhsT=w2[:], rhs=h1[:],
                                     start=True, stop=True)
                    h2 = pw.tile([H1, 128], FP32, tag="h2")
                    nc.scalar.activation(
                        out=h2[:], in_=h2_ps[:],
                        func=act_func,
                        bias=b2[:, 0:1])

                    o_ps = dps.tile([128, 128], FP32, tag="dps")
                    nc.tensor.matmul(out=o_ps[:OUT_F, :], lhsT=w3[:], rhs=h2[:],
                                     start=True, stop=True)
                    o_sb = pw.tile([OUT_F, 128], FP32, tag="osb")
                    nc.vector.tensor_scalar(
                        out=o_sb[:], in0=o_ps[:OUT_F, :],
                        scalar1=b3[:, 0:1], scalar2=None,
                        op0=mybir.AluOpType.add)
                    nc.sync.dma_start(
                        out=out_d[:, w * 128:(w + 1) * 128], in_=o_sb[:])
    if not nc.is_finalized():
        nc.finalize()
    return nc


def kernel(**inputs):
    cores, cw = _host_prep(inputs)
    E = np.asarray(inputs["atomic_embedding"], np.float32)
    q = np.asarray(inputs["partial_charges"], np.float32)[:, 0]
    agh_rs = np.ascontiguousarray(
        np.asarray(inputs["agh"], np.float32).reshape(64, 512))
    W1e = np.ascontiguousarray(np.asarray(inputs["W1"], np.float32)[:IN_EFF])
    W2 = np.asarray(inputs["W2"], np.float32)
    W3 = np.asarray(inputs["W3"], np.float32)
    b1 = np.ascontiguousarray(np.asarray(inputs["b1"], np.float32)[:, None])
    b2 = np.ascontiguousarray(np.asarray(inputs["b2"], np.float32)[:, None])
    b3 = np.ascontiguousarray(np.asarray(inputs["b3"], np.float32)[:, None])
    iota = np.broadcast_to(
        np.arange(128, dtype=np.float32)[None, :, None], (128, 128, max(cw))
    ).astype(ml_dtypes.bfloat16)
    ident = np.eye(128, dtype=np.float32)

    in_maps = []
    for c in range(N_CORES):
        gs_p, gv_p, ix_p = cores[c]
        E_loc = np.zeros((NAP, 64), np.float32)
        E_loc[:NA] = E[c * NA:(c + 1) * NA]
        q_loc = np.zeros((NAP,), np.float32)
        q_loc[:NA] = q[c * NA:(c + 1) * NA]
        elocr = np.ascontiguousarray(
            E_loc.reshape(NW, 128, 64).transpose(1, 0, 2).reshape(128, NW * 64))
        qr = np.ascontiguousarray(q_loc.reshape(NW, 128).T)
        in_maps.append({
            "gsp": gs_p, "gvp": gv_p, "ixp": ix_p,
            "elocr": elocr, "qr": qr, "aghr": agh_rs,
            "w1e": W1e, "w2": W2, "w3": W3,
            "b1c": b1, "b2c": b2, "b3c": b3,
            "iota": iota, "ident": ident,
        })

    nc = _build_nc(cw)
    res = run_bass_kernel_spmd(nc, in_maps, list(range(N_CORES)))
    if getattr(res, "exec_time_ns", None):
        print(f"HW exec time: {res.exec_time_ns} ns")
    outs = [res.results[c]["out"][:, :NA] for c in range(N_CORES)]
    full = np.concatenate(outs, axis=1).T.astype(np.float32)  # [50000, 66]
    delta_q = np.ascontiguousarray(full[:, 0:1])
    f = np.ascontiguousarray(full[:, 1:2])
    delta_a = np.ascontiguousarray(full[:, 2:])
    return delta_a, delta_q, f
